# revision 9
# baseline (speedup 1.0000x reference)
"""GSU (gated spiking unit) Trainium2 kernel.

Reference computation (T=4, B=32, N=512, DIM=512):
    outputs, gate = split(inputs, 2, axis=-1)
    gate = LIF(gate)                  # tau=2 (decay 0.5), v_th=1, hard reset
    gate = gate @ W.T + b             # einsum tbnd,ed->tbne
    gate = LIF(gate)
    out  = outputs * gate

Strategy (8 cores, data-parallel over B, 4 batch elems/core):
  * Host pre-transposes each (t, b) slab to [feature, n] layout so the
    spike matrix lands on-chip already in the matmul contraction layout
    (d on partitions).  No on-chip transposes at all.
  * LIF states are kept scaled by 2^t (exact in fp32: exponent shift),
    which turns every LIF step into fused DVE ops:
        p_t   = (x_t * 2^t) + phat_{t-1}          scalar_tensor_tensor
        m_t   = (p_t < 2^t)                       tensor_scalar (2x mode)
        phat  = (p_t < 2^t) * p_t                 scalar_tensor_tensor
    m is the COMPLEMENT of the spike (s = 1-m), folded into the matmul:
        y*2^t = m @ (-2^t W).T + 2^t (b + rowsum(W)) + uhat_{t-1}
    The uhat carry is added by an identity matmul straight into PSUM;
    the per-partition bias rides on the ScalarE Identity activation that
    also evacuates PSUM -> SBUF.  Final gating is one fused op:
        out = (u >= 2^t) * outputs
  * Matmuls run in float32r (full-rate fp32 on the PE array).
"""

import os
import sys
from contextlib import ExitStack

import numpy as np

sys.path.insert(0, "/opt/trn_rl_repo")

import concourse.bass as bass
import concourse.tile as tile
from concourse import bacc
from concourse import mybir
from concourse._compat import with_exitstack
from concourse.bass_utils import run_bass_kernel_spmd

T, B, N, DIM = 4, 32, 512, 512
NCORES = 8
BL = B // NCORES          # batch elems per core
F32 = mybir.dt.float32
F32R = mybir.dt.float32r
BF16 = mybir.dt.bfloat16
AF = mybir.ActivationFunctionType
OP = mybir.AluOpType

_cache = {}


def build_nc():
    nc = bacc.Bacc(None)
    xg = nc.declare_dram_parameter("xg", [BL, T, DIM, N], F32, isOutput=False)
    xo = nc.declare_dram_parameter("xo", [BL, T, DIM, N], F32, isOutput=False)
    wts = nc.declare_dram_parameter("wts", [3 * T * DIM, DIM], BF16, isOutput=False)
    bias = nc.declare_dram_parameter("bias", [128, T * 4 + T], F32, isOutput=False)
    out = nc.declare_dram_parameter("out", [BL, T, DIM, N], F32, isOutput=True)

    FD = 4 * N  # 2048 free elems per [128, FD] working tile

    with TileKernel(nc) as (ctx, tc):
        const = ctx.enter_context(tc.tile_pool(name="const", bufs=1))
        wt = const.tile([128, 48 * DIM], BF16, tag="wt")
        nc.sync.dma_start(
            out=wt[:].rearrange("p (x e) -> p x e", x=48),
            in_=wts.rearrange("(x p) e -> p x e", p=128),
        )
        bt = const.tile([128, T * 4 + T], F32, tag="bt")
        nc.sync.dma_start(out=bt[:], in_=bias[:, :])

        scratch = const.tile([128, 8], F32, tag="scratch")
        pp = ctx.enter_context(tc.tile_pool(name="pp", bufs=8, space="PSUM"))
        # one-time "touchers": absorb the const-DMA completion waits onto
        # throwaway ops so hot instructions carry at most one sync wait
        # (ISA limit on fused-LDW matmult / activation wait slots).
        dps = pp.tile([1, 8], F32, tag="ps")
        nc.tensor.matmul(
            out=dps[:, :], lhsT=wt[:, 0:1], rhs=wt[:, 0:8], start=True, stop=True
        )
        nc.vector.tensor_copy(scratch[:, 0:1], bt[:, 0:1])

        io = ctx.enter_context(tc.tile_pool(name="io", bufs=3))
        work = ctx.enter_context(tc.tile_pool(name="work", bufs=2))
        state = ctx.enter_context(tc.tile_pool(name="state", bufs=2))

        for b in range(BL):
            phat = state.tile([128, FD], F32, tag="phat")
            uhat = state.tile([128, FD], F32, tag="uhat")
            for t in range(T):
                th = float(2 ** t)
                xgt = io.tile([128, FD], F32, tag="xg")
                nc.sync.dma_start(
                    out=xgt[:].rearrange("p (c n) -> p c n", c=4),
                    in_=xg[b, t].rearrange("(c p) n -> p c n", p=128),
                )
                xot = io.tile([128, FD], F32, tag="xo")
                nc.sync.dma_start(
                    out=xot[:].rearrange("p (c n) -> p c n", c=4),
                    in_=xo[b, t].rearrange("(c p) n -> p c n", p=128),
                )

                # ---- LIF1 (p is the pre-reset membrane scaled by 2^t) ----
                if t == 0:
                    p = xgt
                else:
                    p = work.tile([128, FD], F32, tag="p")
                    nc.vector.scalar_tensor_tensor(
                        out=p[:], in0=xgt[:], scalar=th, in1=phat[:],
                        op0=OP.mult, op1=OP.add,
                    )
                m = work.tile([128, FD], BF16, tag="m")
                nc.vector.tensor_scalar(
                    out=m[:], in0=p[:], scalar1=th, scalar2=None, op0=OP.is_lt
                )
                if t < T - 1:
                    nc.vector.scalar_tensor_tensor(
                        out=phat[:], in0=p[:], scalar=th, in1=p[:],
                        op0=OP.is_lt, op1=OP.mult,
                    )

                # ---- Linear (+ uhat carry) into PSUM, float32r ----
                u = work.tile([128, FD], F32, tag="u")
                for ec in range(4):
                    ps = pp.tile([128, N], F32, tag="ps")
                    nmm = 0
                    for term in range(3):
                        for dc in range(4):
                            x = (term * 16 + t * 4 + dc) * DIM + ec * 128
                            nmm += 1
                            nc.tensor.matmul(
                                out=ps[:],
                                lhsT=wt[:, x:x + 128],
                                rhs=m[:, dc * N:(dc + 1) * N],
                                start=(nmm == 1),
                                stop=(nmm == 12),
                            )
                    # ---- LIF2 charge: u = psum + bias(+ carry) ----
                    bslice = bt[:, t * 4 + ec:t * 4 + ec + 1]
                    if t == 0:
                        nc.vector.tensor_scalar(
                            out=u[:, ec * N:(ec + 1) * N], in0=ps[:],
                            scalar1=bslice, scalar2=None, op0=OP.add,
                        )
                    else:
                        nc.vector.scalar_tensor_tensor(
                            out=u[:, ec * N:(ec + 1) * N], in0=ps[:],
                            scalar=bslice, in1=uhat[:, ec * N:(ec + 1) * N],
                            op0=OP.add, op1=OP.add,
                        )

                # ---- gate + output ----
                res = io.tile([128, FD], F32, tag="res")
                nc.vector.scalar_tensor_tensor(
                    out=res[:], in0=u[:], scalar=th, in1=xot[:],
                    op0=OP.is_ge, op1=OP.mult,
                )
                if t < T - 1:
                    nc.vector.scalar_tensor_tensor(
                        out=uhat[:], in0=u[:], scalar=th, in1=u[:],
                        op0=OP.is_lt, op1=OP.mult,
                    )
                nc.sync.dma_start(
                    out=out[b, t].rearrange("(c p) n -> p c n", p=128),
                    in_=res[:].rearrange("p (c n) -> p c n", c=4),
                )
    nc.compile()
    return nc


class TileKernel:
    def __init__(self, nc):
        self.nc = nc
        self.ctx = ExitStack()

    def __enter__(self):
        self.tc = self.ctx.enter_context(tile.TileContext(self.nc))
        return self.ctx, self.tc

    def __exit__(self, *a):
        return self.ctx.__exit__(*a)


def host_prep(inputs, W, b):
    inputs = np.asarray(inputs, dtype=np.float32)
    W = np.asarray(W, dtype=np.float32)
    b = np.asarray(b, dtype=np.float32)

    scal = (2.0 ** np.arange(T)).astype(np.float32)
    # lhsT[d, e] = (-2^t * W)[e, d]
    import ml_dtypes
    wf = np.ascontiguousarray(
        (-W.T)[None, :, :] * scal[:, None, None]
    ).reshape(T * DIM, DIM).astype(np.float32)
    w_hi = wf.astype(ml_dtypes.bfloat16)
    w_mid = (wf - w_hi.astype(np.float32)).astype(ml_dtypes.bfloat16)
    w_lo = (wf - w_hi.astype(np.float32) - w_mid.astype(np.float32)).astype(
        ml_dtypes.bfloat16
    )
    wts = np.ascontiguousarray(np.stack([w_hi, w_mid, w_lo]).reshape(3 * T * DIM, DIM))
    bc = (b[None, :] + W.sum(axis=1)[None, :]) * scal[:, None]   # [T, DIM]
    bias = np.concatenate([
        bc.reshape(T, 4, 128).transpose(2, 0, 1).reshape(128, T * 4),
        np.broadcast_to(-scal, (128, T)),
    ], axis=1).astype(np.float32)
    bias = np.ascontiguousarray(bias)

    in_maps = []
    for c in range(NCORES):
        sl = inputs[:, c * BL:(c + 1) * BL]          # [T, BL, N, 2*DIM]
        xo = np.ascontiguousarray(sl[..., :DIM].transpose(1, 0, 3, 2))
        xg = np.ascontiguousarray(sl[..., DIM:].transpose(1, 0, 3, 2))
        in_maps.append({"xg": xg, "xo": xo, "wts": wts, "bias": bias})
    return in_maps


def host_post(results):
    full = np.empty((T, B, N, DIM), dtype=np.float32)
    for c, r in enumerate(results):
        # r["out"]: [BL, T, DIM(e), N] -> [T, BL, N, DIM]
        full[:, c * BL:(c + 1) * BL] = r["out"].transpose(1, 0, 3, 2)
    return full


def run(inputs, W, b, trace=False, **trace_kw):
    in_maps = host_prep(inputs, W, b)
    if "nc" not in _cache:
        _cache["nc"] = build_nc()
    br = run_bass_kernel_spmd(
        _cache["nc"], in_maps, core_ids=list(range(NCORES)), trace=trace, **trace_kw
    )
    return host_post(br.results), br


def kernel(inputs, W, b):
    out, _ = run(inputs, W, b, trace=False)
    return out
